# revision 1
# baseline (speedup 1.0000x reference)
"""Trainium2 Bass kernel for DiagonalGMMPosterior (vq_codebook).

Reference computation (per batch b, descriptor n, cluster k):
    dist[k,n]  = sum_d (x[d,n] - mu_n[k,d])^2 * exp(-log_sigma[k,d])
    logits     = -dist + log_alpha[k] - 0.5 * sum_d log_sigma[k,d]
    out[k,n]   = softmax_k(logits)

Device strategy (8 NeuronCores, data-parallel over the batch axis):
  * Host folds all (K,D) parameter math into two GEMM weight matrices and a
    per-cluster constant, then CENTERS them across K.  Softmax is invariant
    to per-n shifts, so subtracting the K-mean of the logits (a rank-1
    update folded into the weights on host) bounds the logits to ~±16 and
    removes the need for a per-n max reduction entirely — no transposes,
    no partition-axis max.
  * Per core: for each 1024-column tile of x (D=128 on partitions, two
    512-wide PSUM banks side by side; float32r streams fp32 through the
    PE at 1 cycle/row instead of 4):
       psum = W1^T @ x^2 + W2^T @ x          (TensorE, PSUM accumulate)
       e    = exp(psum + cc)                 (ScalarE, per-partition bias)
       s    = ones_K^T @ e                   (TensorE: partition-sum)
       r    = 1/s                            (VectorE)
       rb   = ones_1K^T @ r                  (TensorE: partition-broadcast)
       out  = e * rb                         (VectorE)
"""

import numpy as np

import concourse.bacc as bacc
import concourse.bass as bass
import concourse.tile as tile
from concourse import mybir
from concourse.bass_utils import run_bass_kernel_spmd

B, D, N, K = 16, 128, 16384, 64
NCORES = 8
BPC = B // NCORES  # batches per core
NT = 512           # one PSUM bank of fp32
PAIR = 2 * NT      # two banks processed per iteration

F32 = mybir.dt.float32
F32R = mybir.dt.float32r
F16 = mybir.dt.float16

_CACHE = {}


def _build_nc():
    # Bacc (not raw Bass): its compile() pass legalizes Tile's multi-wait
    # instructions (move_matmul_waits_to_ldweights + generate_event_semaphores)
    # down to the 1-wait-per-instruction hardware limit.
    nc = bacc.Bacc("TRN2", target_bir_lowering=False, debug=False)
    x_in = nc.declare_dram_parameter("x", [BPC, D, N], F32R, isOutput=False)
    w1_in = nc.declare_dram_parameter("w1", [D, K], F32R, isOutput=False)
    w2_in = nc.declare_dram_parameter("w2", [D, K], F32R, isOutput=False)
    cc_in = nc.declare_dram_parameter("cc", [K, 1], F32, isOutput=False)
    ones_in = nc.declare_dram_parameter("ones_kk", [K, K], F32R, isOutput=False)
    # fp16 output halves the store traffic at ~5e-4 rounding (posteriors
    # live in [0,1], well inside fp16 range); the host widens back to fp32
    out_ext = nc.declare_dram_parameter("out", [BPC, K, N], F16, isOutput=True)

    with tile.TileContext(nc) as tc:
        with (
            tc.tile_pool(name="consts", bufs=1) as consts,
            tc.tile_pool(name="xp", bufs=8) as xp,
            tc.tile_pool(name="ep", bufs=6) as ep,
            tc.tile_pool(name="op", bufs=6) as op,
            tc.tile_pool(name="rp", bufs=6) as rp,
            tc.tile_pool(name="pd", bufs=2, space="PSUM") as pdp,
            tc.tile_pool(name="pb", bufs=2, space="PSUM") as pbp,
        ):
            w1_sb = consts.tile([D, K], F32R)
            nc.sync.dma_start(out=w1_sb, in_=w1_in[:, :])
            w2_sb = consts.tile([D, K], F32R)
            nc.sync.dma_start(out=w2_sb, in_=w2_in[:, :])
            cc_sb = consts.tile([K, 1], F32)
            nc.sync.dma_start(out=cc_sb, in_=cc_in[:, :])
            ones_kk = consts.tile([K, K], F32R)
            nc.sync.dma_start(out=ones_kk, in_=ones_in[:, :])

            n_pairs = N // PAIR  # 16 per batch row
            pairs = [(b, p) for b in range(BPC) for p in range(n_pairs)]
            NP = len(pairs)
            st = [dict() for _ in range(NP)]

            # software-pipelined emission: each engine's in-order stream
            # interleaves stages of consecutive pairs so no stage
            # head-of-line-blocks the next pair's earlier stage
            def s0_load(i):
                b, p = pairs[i]
                n0 = p * PAIR
                xt = xp.tile([D, PAIR], F32R, tag="xt")
                nc.sync.dma_start(out=xt, in_=x_in[b, :, n0 : n0 + PAIR])
                st[i]["xt"] = xt

            def s1_square(i):
                xt = st[i]["xt"]
                xsq = xp.tile([D, PAIR], F32R, tag="xsq")
                nc.scalar.activation(
                    out=xsq, in_=xt.bitcast(F32),
                    func=mybir.ActivationFunctionType.Square,
                )
                st[i]["xsq"] = xsq

            def s2_dist(i):
                xt, xsq = st[i]["xt"], st[i]["xsq"]
                # dist-difference GEMM: two 512-wide halves, one PSUM
                # bank each, both at base partition 0 (f32r matmuls
                # reject other output base partitions)
                pd_t = pdp.tile([K, PAIR], F32, tag="pd")
                for h in range(2):
                    sl = slice(h * NT, (h + 1) * NT)
                    nc.tensor.matmul(
                        pd_t[:, sl], w1_sb[:, :], xsq[:, sl],
                        start=True, stop=False,
                    )
                    nc.tensor.matmul(
                        pd_t[:, sl], w2_sb[:, :], xt[:, sl],
                        start=False, stop=True,
                    )
                st[i]["pd"] = pd_t

            def s3_exp(i):
                pd_t = st[i].pop("pd")
                et = ep.tile([K, PAIR], F32R, tag="et")
                nc.scalar.activation(
                    out=et, in_=pd_t,
                    func=mybir.ActivationFunctionType.Exp,
                    bias=cc_sb, scale=1.0,
                )
                st[i]["et"] = et
                st[i].pop("xt")
                st[i].pop("xsq")

            def s4_den(i):
                et = st[i]["et"]
                # denominator, summed over k AND broadcast to all 64
                # partitions in one shot: ones_kk^T @ et
                pb_t = pbp.tile([K, PAIR], F32, tag="pb")
                for h in range(2):
                    sl = slice(h * NT, (h + 1) * NT)
                    nc.tensor.matmul(
                        pb_t[:, sl], ones_kk[:, :], et[:, sl],
                        start=True, stop=True,
                    )
                st[i]["pb"] = pb_t

            def s5_recip(i):
                pb_t = st[i].pop("pb")
                r_all = rp.tile([K, PAIR], F32, tag="r")
                # ~18-bit-accurate custom-DVE reciprocal, ~5x faster than
                # the exact iterative-divide reciprocal(); the sum is
                # always >= 1 (mean-centered logits), so the undefined
                # edge cases (0/denorm/inf) cannot occur
                nc.vector.reciprocal_approx_fast(out=r_all, in_=pb_t)
                st[i]["r"] = r_all

            def s6_mult(i):
                et, r_all = st[i].pop("et"), st[i].pop("r")
                ot = op.tile([K, PAIR], F16, tag="ot")
                nc.vector.tensor_mul(ot, et.bitcast(F32), r_all)
                st[i]["ot"] = ot

            def s7_store(i):
                b, p = pairs[i]
                n0 = p * PAIR
                ot = st[i].pop("ot")
                nc.sync.dma_start(
                    out=out_ext[b, :, n0 : n0 + PAIR], in_=ot[:, :]
                )

            stages = [
                s0_load, s1_square, s2_dist, s3_exp,
                s4_den, s5_recip, s6_mult, s7_store,
            ]
            NS = len(stages)
            # downstream stages emitted first within each tick so no
            # engine's in-order queue blocks a later pair's earlier stage
            for tick in range(NP + NS - 1):
                for k in reversed(range(NS)):
                    i = tick - k
                    if 0 <= i < NP:
                        stages[k](i)
    nc.compile()
    return nc


def _host_params(mu, log_sigma, log_alpha):
    mu64 = mu.astype(np.float64)
    mu_n = mu64 / np.maximum(
        np.linalg.norm(mu64, axis=1, keepdims=True), 1e-12
    )
    sinv = np.exp(-log_sigma.astype(np.float64))  # (K, D)
    a1 = -sinv                                    # coeff of x^2 in logits
    a2 = 2.0 * mu_n * sinv                        # coeff of x
    c = (
        -np.sum(mu_n * mu_n * sinv, axis=1)
        + log_alpha.astype(np.float64)
        - 0.5 * np.sum(log_sigma.astype(np.float64), axis=1)
    )
    # center across K: softmax is invariant to per-n shifts, and this keeps
    # the on-device logits within exp()'s comfortable fp32 range (~±16)
    a1c = a1 - a1.mean(axis=0, keepdims=True)
    a2c = a2 - a2.mean(axis=0, keepdims=True)
    ccv = c - c.mean()
    w1 = np.ascontiguousarray(a1c.T, dtype=np.float32)  # (D, K)
    w2 = np.ascontiguousarray(a2c.T, dtype=np.float32)  # (D, K)
    cc = ccv.astype(np.float32).reshape(K, 1)
    return w1, w2, cc


def _in_maps(x, mu, log_sigma, log_alpha):
    x = np.ascontiguousarray(np.asarray(x), dtype=np.float32)
    w1, w2, cc = _host_params(
        np.asarray(mu), np.asarray(log_sigma), np.asarray(log_alpha)
    )
    ones_kk = np.ones((K, K), dtype=np.float32)
    return [
        {
            "x": x[i * BPC : (i + 1) * BPC],
            "w1": w1,
            "w2": w2,
            "cc": cc,
            "ones_kk": ones_kk,
        }
        for i in range(NCORES)
    ]


def kernel(x, mu, log_sigma, log_alpha):
    if "nc" not in _CACHE:
        _CACHE["nc"] = _build_nc()
    nc = _CACHE["nc"]
    in_maps = _in_maps(x, mu, log_sigma, log_alpha)
    res = run_bass_kernel_spmd(nc, in_maps, list(range(NCORES))).results
    out = np.concatenate(
        [np.asarray(res[i]["out"]) for i in range(NCORES)], axis=0
    )
    return out.astype(np.float32)



# revision 2
# speedup vs baseline: 1.2982x; 1.2982x over previous
"""Trainium2 Bass kernel for DiagonalGMMPosterior (vq_codebook).

Reference computation (per batch b, descriptor n, cluster k):
    dist[k,n]  = sum_d (x[d,n] - mu_n[k,d])^2 * exp(-log_sigma[k,d])
    logits     = -dist + log_alpha[k] - 0.5 * sum_d log_sigma[k,d]
    out[k,n]   = softmax_k(logits)

Device strategy (8 NeuronCores, data-parallel over the batch axis):
  * Host folds all (K,D) parameter math into two GEMM weight matrices and a
    per-cluster constant, then CENTERS them across K.  Softmax is invariant
    to per-n shifts, so subtracting the K-mean of the logits (a rank-1
    update folded into the weights on host) bounds the logits and removes
    the need for a per-n max reduction entirely.
  * x is shipped to the device as fp16 (halves HBM load traffic).  The
    k-dependent part of the fp16 rounding error is ~1e-3 in logit space
    (the k-independent part cancels in softmax), far inside the 2e-2 gate.
  * K=64 but SBUF/PSUM have 128 partitions, so two 512-column blocks are
    STACKED: block A's logits land on PSUM partitions 0-63 (PE column tile
    0) and block B's on 64-127 (PE column tile 64).  Every post-GEMM op
    (exp / ones-matmul / reciprocal / multiply) then runs on 128 partitions
    at half the free size — half the engine time of the flat layout.
  * Per core, per 1024-column tile of x:
       xsq  = x * x                          (VectorE, fp16)
       psum[0:64,:]   = w1^T xsq_A + w2^T x_A   (TensorE, fp16)
       psum[64:128,:] = w1^T xsq_B + w2^T x_B   (TensorE, fp16)
       e    = exp(psum + cc)                 (ScalarE, bias per partition,
                                              bf16 out)
       s    = ones_bd^T @ e                  (TensorE: block-diag ones sums
                                              each 64-partition block AND
                                              broadcasts, one 512-col pass)
       r    = 1/s                            (VectorE, approx reciprocal)
       out  = e * r                          (VectorE, fp16 out)
"""

import numpy as np

import concourse.bacc as bacc
import concourse.bass as bass
import concourse.tile as tile
from concourse import mybir
from concourse.bass_utils import run_bass_kernel_spmd

B, D, N, K = 16, 128, 16384, 64
NCORES = 8
BPC = B // NCORES  # batches per core
NT = 512           # one PSUM bank of fp32; stacked block width
PAIR = 2 * NT      # columns of x per iteration (two stacked blocks)

F32 = mybir.dt.float32
F16 = mybir.dt.float16
BF16 = mybir.dt.bfloat16

_CACHE = {}


def _build_nc():
    # Bacc (not raw Bass): its compile() pass legalizes Tile's multi-wait
    # instructions down to the 1-wait-per-instruction hardware limit.
    nc = bacc.Bacc("TRN2", target_bir_lowering=False, debug=False)
    x_in = nc.declare_dram_parameter("x", [BPC, D, N], F16, isOutput=False)
    w1_in = nc.declare_dram_parameter("w1", [D, K], F16, isOutput=False)
    w2_in = nc.declare_dram_parameter("w2", [D, K], F16, isOutput=False)
    cc_in = nc.declare_dram_parameter("cc", [2 * K, 1], F32, isOutput=False)
    ones_in = nc.declare_dram_parameter("ones_bd", [2 * K, 2 * K], BF16,
                                        isOutput=False)
    # fp16 output halves the store traffic at ~5e-4 rounding (posteriors
    # live in [0,1]); the host widens back to fp32
    out_ext = nc.declare_dram_parameter("out", [BPC, K, N], F16, isOutput=True)

    with tile.TileContext(nc) as tc:
        with (
            tc.tile_pool(name="consts", bufs=1) as consts,
            tc.tile_pool(name="xp", bufs=6) as xp,
            tc.tile_pool(name="qp", bufs=6) as qp,
            tc.tile_pool(name="ep", bufs=6) as ep,
            tc.tile_pool(name="op", bufs=6) as op,
            tc.tile_pool(name="rp", bufs=6) as rp,
            tc.tile_pool(name="pd", bufs=4, space="PSUM") as pdp,
            tc.tile_pool(name="pb", bufs=4, space="PSUM") as pbp,
        ):
            w1_sb = consts.tile([D, K], F16)
            nc.sync.dma_start(out=w1_sb, in_=w1_in[:, :])
            w2_sb = consts.tile([D, K], F16)
            nc.sync.dma_start(out=w2_sb, in_=w2_in[:, :])
            cc_sb = consts.tile([2 * K, 1], F32)
            nc.sync.dma_start(out=cc_sb, in_=cc_in[:, :])
            ones_bd = consts.tile([2 * K, 2 * K], BF16)
            nc.sync.dma_start(out=ones_bd, in_=ones_in[:, :])

            n_pairs = N // PAIR  # 16 per batch row
            pairs = [(b, p) for b in range(BPC) for p in range(n_pairs)]
            NP = len(pairs)
            st = [dict() for _ in range(NP)]

            # software-pipelined emission: each engine's in-order stream
            # interleaves stages of consecutive pairs so no stage
            # head-of-line-blocks the next pair's earlier stage
            def s0_load(i):
                b, p = pairs[i]
                n0 = p * PAIR
                xt = xp.tile([D, PAIR], F16, tag="xt")
                nc.sync.dma_start(out=xt, in_=x_in[b, :, n0 : n0 + PAIR])
                st[i]["xt"] = xt

            def s1_square(i):
                xt = st[i]["xt"]
                xsq = qp.tile([D, PAIR], F16, tag="xsq")
                nc.vector.tensor_mul(xsq, xt, xt)
                st[i]["xsq"] = xsq

            def s2_dist(i):
                xt, xsq = st[i]["xt"], st[i]["xsq"]
                # stacked dist GEMM: block A (cols 0:512) accumulates on
                # PSUM partitions 0:64 (PE column tile 0), block B
                # (cols 512:1024) on partitions 64:128 (PE column tile 64)
                pd_t = pdp.tile([2 * K, NT], F32, tag="pd")
                for h in range(2):
                    sl = slice(h * NT, (h + 1) * NT)
                    pr = slice(h * K, (h + 1) * K)
                    nc.tensor.matmul(
                        pd_t[pr, :], w1_sb[:, :], xsq[:, sl],
                        start=True, stop=False,
                    )
                    nc.tensor.matmul(
                        pd_t[pr, :], w2_sb[:, :], xt[:, sl],
                        start=False, stop=True,
                    )
                st[i]["pd"] = pd_t

            def s3_exp(i):
                pd_t = st[i].pop("pd")
                et = ep.tile([2 * K, NT], BF16, tag="et")
                nc.scalar.activation(
                    out=et, in_=pd_t,
                    func=mybir.ActivationFunctionType.Exp,
                    bias=cc_sb, scale=1.0,
                )
                st[i]["et"] = et
                st[i].pop("xt")
                st[i].pop("xsq")

            def s4_den(i):
                et = st[i]["et"]
                # denominator: block-diag ones sums each 64-partition block
                # separately AND broadcasts the sum to all 64 partitions of
                # that block, in a single 512-column stream
                pb_t = pbp.tile([2 * K, NT], F32, tag="pb")
                nc.tensor.matmul(
                    pb_t[:, :], ones_bd[:, :], et[:, :],
                    start=True, stop=True,
                )
                st[i]["pb"] = pb_t

            def s5_recip(i):
                pb_t = st[i].pop("pb")
                r_all = rp.tile([2 * K, NT], F32, tag="r")
                # ~18-bit-accurate custom-DVE reciprocal; the sum is always
                # >= 1 (mean-centered logits), so the undefined edge cases
                # (0/denorm/inf) cannot occur
                nc.vector.reciprocal_approx_fast(out=r_all, in_=pb_t)
                st[i]["r"] = r_all

            def s6_mult(i):
                et, r_all = st[i].pop("et"), st[i].pop("r")
                ot = op.tile([2 * K, NT], F16, tag="ot")
                nc.vector.tensor_mul(ot, et, r_all)
                st[i]["ot"] = ot

            def s7_store(i):
                b, p = pairs[i]
                n0 = p * PAIR
                ot = st[i].pop("ot")
                for h in range(2):
                    nc.sync.dma_start(
                        out=out_ext[b, :, n0 + h * NT : n0 + (h + 1) * NT],
                        in_=ot[h * K : (h + 1) * K, :],
                    )

            stages = [
                s0_load, s1_square, s2_dist, s3_exp,
                s4_den, s5_recip, s6_mult, s7_store,
            ]
            NS = len(stages)
            # downstream stages emitted first within each tick so no
            # engine's in-order queue blocks a later pair's earlier stage
            for tick in range(NP + NS - 1):
                for k in reversed(range(NS)):
                    i = tick - k
                    if 0 <= i < NP:
                        stages[k](i)
    nc.compile()
    return nc


def _host_params(mu, log_sigma, log_alpha):
    mu64 = mu.astype(np.float64)
    mu_n = mu64 / np.maximum(
        np.linalg.norm(mu64, axis=1, keepdims=True), 1e-12
    )
    sinv = np.exp(-log_sigma.astype(np.float64))  # (K, D)
    a1 = -sinv                                    # coeff of x^2 in logits
    a2 = 2.0 * mu_n * sinv                        # coeff of x
    c = (
        -np.sum(mu_n * mu_n * sinv, axis=1)
        + log_alpha.astype(np.float64)
        - 0.5 * np.sum(log_sigma.astype(np.float64), axis=1)
    )
    # center across K: softmax is invariant to per-n shifts, and this keeps
    # the on-device logits within exp()'s comfortable range
    a1c = a1 - a1.mean(axis=0, keepdims=True)
    a2c = a2 - a2.mean(axis=0, keepdims=True)
    ccv = c - c.mean()
    w1 = np.ascontiguousarray(a1c.T, dtype=np.float16)  # (D, K)
    w2 = np.ascontiguousarray(a2c.T, dtype=np.float16)  # (D, K)
    cc = np.tile(ccv.astype(np.float32).reshape(K, 1), (2, 1))  # (2K, 1)
    return w1, w2, cc


def _in_maps(x, mu, log_sigma, log_alpha):
    x = np.asarray(x).astype(np.float16)
    w1, w2, cc = _host_params(
        np.asarray(mu), np.asarray(log_sigma), np.asarray(log_alpha)
    )
    from ml_dtypes import bfloat16
    ones_bd = np.kron(
        np.eye(2, dtype=np.float32), np.ones((K, K), dtype=np.float32)
    ).astype(bfloat16)
    return [
        {
            "x": np.ascontiguousarray(x[i * BPC : (i + 1) * BPC]),
            "w1": w1,
            "w2": w2,
            "cc": cc,
            "ones_bd": ones_bd,
        }
        for i in range(NCORES)
    ]


def kernel(x, mu, log_sigma, log_alpha):
    if "nc" not in _CACHE:
        _CACHE["nc"] = _build_nc()
    nc = _CACHE["nc"]
    in_maps = _in_maps(x, mu, log_sigma, log_alpha)
    res = run_bass_kernel_spmd(nc, in_maps, list(range(NCORES))).results
    out = np.concatenate(
        [np.asarray(res[i]["out"]) for i in range(NCORES)], axis=0
    )
    return out.astype(np.float32)


# revision 3
# speedup vs baseline: 1.5297x; 1.1783x over previous
"""Trainium2 Bass kernel for DiagonalGMMPosterior (vq_codebook).

Reference computation (per batch b, descriptor n, cluster k):
    dist[k,n]  = sum_d (x[d,n] - mu_n[k,d])^2 * exp(-log_sigma[k,d])
    logits     = -dist + log_alpha[k] - 0.5 * sum_d log_sigma[k,d]
    out[k,n]   = softmax_k(logits)

Device strategy (8 NeuronCores, data-parallel over the batch axis):
  * Host folds all (K,D) parameter math into two GEMM weight matrices and a
    per-cluster constant, then CENTERS them across K.  Softmax is invariant
    to per-n shifts, so subtracting the K-mean of the logits (a rank-1
    update folded into the weights on host) bounds the logits and removes
    the need for a per-n max reduction entirely.
  * x is shipped to the device as fp16 (halves HBM load traffic).  The
    k-dependent part of the fp16 rounding error is ~1e-3 in logit space
    (the k-independent part cancels in softmax), far inside the 2e-2 gate.
  * K=64 but SBUF/PSUM have 128 partitions, so consecutive 512-column
    blocks are STACKED: even blocks' logits land on PSUM partitions 0-63
    (PE column tile 0), odd blocks' on 64-127 (PE column tile 64).  Every
    post-GEMM op (exp / ones-matmul / reciprocal / multiply) then runs on
    128 partitions at half the free size.
  * Work is grouped two 1024-column tiles at a time over a two-bank
    [128,1024] PSUM tile, with the 10 matmuls of a group ordered so each
    of the 5 weight tiles (w1@col0, w1@col64, w2@col0, w2@col64, ones) is
    loaded once and the second matmul reuses it via ldweights=False.
  * Engine balance: square on VectorE (fp16, 2x mode), exp on ScalarE
    (bf16 out), denominator block-diag ones-matmul on TensorE (sums each
    64-partition block AND broadcasts in one pass), reciprocal on VectorE,
    final multiply on the otherwise-idle GpSimd engine.
  * The device writes the stacked layout verbatim ([128, N/2] per batch);
    the host de-interleaves with one numpy transpose.  Input DMAs issue on
    the Sync queue, output DMAs on the Scalar queue.
"""

import numpy as np

import concourse.bacc as bacc
import concourse.bass as bass
import concourse.tile as tile
from concourse import mybir
from concourse.bass_utils import run_bass_kernel_spmd

B, D, N, K = 16, 128, 16384, 64
NCORES = 8
BPC = B // NCORES   # batches per core
NT = 512            # one PSUM bank of fp32; stacked block width
GROUP = 4 * NT      # x columns per group (two stacked 1024-col tiles)
NG = N // GROUP     # groups per batch row

F32 = mybir.dt.float32
F16 = mybir.dt.float16
BF16 = mybir.dt.bfloat16

_CACHE = {}


def _build_nc():
    # Bacc (not raw Bass): its compile() pass legalizes Tile's multi-wait
    # instructions down to the 1-wait-per-instruction hardware limit.
    nc = bacc.Bacc("TRN2", target_bir_lowering=False, debug=False)
    x_in = nc.declare_dram_parameter("x", [BPC, D, N], F16, isOutput=False)
    w1_in = nc.declare_dram_parameter("w1", [D, K], F16, isOutput=False)
    w2_in = nc.declare_dram_parameter("w2", [D, K], F16, isOutput=False)
    cc_in = nc.declare_dram_parameter("cc", [2 * K, 1], F32, isOutput=False)
    ones_in = nc.declare_dram_parameter("ones_bd", [2 * K, 2 * K], BF16,
                                        isOutput=False)
    # stacked device layout: partition 64h+k, column 1024g+512p+c holds
    # posterior[k, 2048g+1024p+512h+c]; the host de-interleaves.  fp16
    # halves the store traffic (~5e-4 rounding on values in [0,1]).
    out_ext = nc.declare_dram_parameter("out", [BPC, 2 * K, N // 2], F16,
                                        isOutput=True)

    with tile.TileContext(nc) as tc:
        with (
            tc.tile_pool(name="consts", bufs=1) as consts,
            tc.tile_pool(name="xp", bufs=4) as xp,
            tc.tile_pool(name="qp", bufs=4) as qp,
            tc.tile_pool(name="ep", bufs=4) as ep,
            tc.tile_pool(name="op", bufs=4) as op,
            tc.tile_pool(name="rp", bufs=4) as rp,
            tc.tile_pool(name="pd", bufs=2, space="PSUM") as pdp,
            tc.tile_pool(name="pb", bufs=2, space="PSUM") as pbp,
        ):
            w1_sb = consts.tile([D, K], F16)
            nc.sync.dma_start(out=w1_sb, in_=w1_in[:, :])
            w2_sb = consts.tile([D, K], F16)
            nc.sync.dma_start(out=w2_sb, in_=w2_in[:, :])
            cc_sb = consts.tile([2 * K, 1], F32)
            nc.sync.dma_start(out=cc_sb, in_=cc_in[:, :])
            ones_bd = consts.tile([2 * K, 2 * K], BF16)
            nc.sync.dma_start(out=ones_bd, in_=ones_in[:, :])

            groups = [(b, g) for b in range(BPC) for g in range(NG)]
            NPG = len(groups)
            st = [dict() for _ in range(NPG)]

            def s0_load(i):
                b, g = groups[i]
                n0 = g * GROUP
                xt = xp.tile([D, GROUP], F16, tag="xt")
                nc.sync.dma_start(out=xt, in_=x_in[b, :, n0 : n0 + GROUP])
                st[i]["xt"] = xt

            def s1_square(i):
                xt = st[i]["xt"]
                xsq = qp.tile([D, GROUP], F16, tag="xsq")
                nc.vector.tensor_mul(xsq, xt, xt)
                st[i]["xsq"] = xsq

            def s2_dist(i):
                xt, xsq = st[i]["xt"], st[i]["xsq"]
                # two stacked 1024-col tiles over one two-bank PSUM tile:
                # x columns 1024p + 512h + c -> PSUM partition block h,
                # column 512p + c.  Matmuls are grouped per weight tile so
                # the second matmul reuses the loaded weights.
                pd_t = pdp.tile([2 * K, 2 * NT], F32, tag="pd")
                for w_sb, mv, start in ((w1_sb, xsq, True), (w2_sb, xt, False)):
                    for h in range(2):
                        pr = slice(h * K, (h + 1) * K)
                        for p in range(2):
                            mm = nc.tensor.matmul(
                                pd_t[pr, p * NT : (p + 1) * NT],
                                w_sb[:, :],
                                mv[:, (2 * p + h) * NT : (2 * p + h + 1) * NT],
                                start=start, stop=not start,
                            )
                            if p == 1:
                                mm.ldweights = False
                st[i]["pd"] = pd_t

            def s3_exp(i):
                pd_t = st[i].pop("pd")
                et = ep.tile([2 * K, 2 * NT], BF16, tag="et")
                nc.scalar.activation(
                    out=et, in_=pd_t,
                    func=mybir.ActivationFunctionType.Exp,
                    bias=cc_sb, scale=1.0,
                )
                st[i]["et"] = et
                st[i].pop("xt")
                st[i].pop("xsq")

            def s4_den(i):
                et = st[i]["et"]
                # denominator: block-diag ones sums each 64-partition block
                # separately AND broadcasts the sum to all 64 partitions of
                # that block; two 512-col streams share one weight load
                pb_t = pbp.tile([2 * K, 2 * NT], F32, tag="pb")
                for p in range(2):
                    mm = nc.tensor.matmul(
                        pb_t[:, p * NT : (p + 1) * NT],
                        ones_bd[:, :],
                        et[:, p * NT : (p + 1) * NT],
                        start=True, stop=True,
                    )
                    if p == 1:
                        mm.ldweights = False
                st[i]["pb"] = pb_t

            def s5_recip(i):
                pb_t = st[i].pop("pb")
                r_all = rp.tile([2 * K, 2 * NT], F32, tag="r")
                # ~18-bit-accurate custom-DVE reciprocal; the sum is always
                # >= 1 (mean-centered logits), so the undefined edge cases
                # (0/denorm/inf) cannot occur
                nc.vector.reciprocal_approx_fast(out=r_all, in_=pb_t)
                st[i]["r"] = r_all

            def s6_mult(i):
                et, r_all = st[i].pop("et"), st[i].pop("r")
                ot = op.tile([2 * K, 2 * NT], F16, tag="ot")
                nc.gpsimd.tensor_mul(ot, et, r_all)
                st[i]["ot"] = ot

            def s7_store(i):
                b, g = groups[i]
                c0 = g * 2 * NT
                ot = st[i].pop("ot")
                nc.scalar.dma_start(
                    out=out_ext[b, :, c0 : c0 + 2 * NT], in_=ot[:, :]
                )

            stages = [
                s0_load, s1_square, s2_dist, s3_exp,
                s4_den, s5_recip, s6_mult, s7_store,
            ]
            NS = len(stages)
            # downstream stages emitted first within each tick so no
            # engine's in-order queue blocks a later group's earlier stage
            for tick in range(NPG + NS - 1):
                for k in reversed(range(NS)):
                    i = tick - k
                    if 0 <= i < NPG:
                        stages[k](i)
    nc.compile()
    return nc


def _host_params(mu, log_sigma, log_alpha):
    mu64 = mu.astype(np.float64)
    mu_n = mu64 / np.maximum(
        np.linalg.norm(mu64, axis=1, keepdims=True), 1e-12
    )
    sinv = np.exp(-log_sigma.astype(np.float64))  # (K, D)
    a1 = -sinv                                    # coeff of x^2 in logits
    a2 = 2.0 * mu_n * sinv                        # coeff of x
    c = (
        -np.sum(mu_n * mu_n * sinv, axis=1)
        + log_alpha.astype(np.float64)
        - 0.5 * np.sum(log_sigma.astype(np.float64), axis=1)
    )
    # center across K: softmax is invariant to per-n shifts, and this keeps
    # the on-device logits within exp()'s comfortable range
    a1c = a1 - a1.mean(axis=0, keepdims=True)
    a2c = a2 - a2.mean(axis=0, keepdims=True)
    ccv = c - c.mean()
    w1 = np.ascontiguousarray(a1c.T, dtype=np.float16)  # (D, K)
    w2 = np.ascontiguousarray(a2c.T, dtype=np.float16)  # (D, K)
    cc = np.tile(ccv.astype(np.float32).reshape(K, 1), (2, 1))  # (2K, 1)
    return w1, w2, cc


def _in_maps(x, mu, log_sigma, log_alpha):
    x = np.asarray(x).astype(np.float16)
    w1, w2, cc = _host_params(
        np.asarray(mu), np.asarray(log_sigma), np.asarray(log_alpha)
    )
    from ml_dtypes import bfloat16
    ones_bd = np.kron(
        np.eye(2, dtype=np.float32), np.ones((K, K), dtype=np.float32)
    ).astype(bfloat16)
    return [
        {
            "x": np.ascontiguousarray(x[i * BPC : (i + 1) * BPC]),
            "w1": w1,
            "w2": w2,
            "cc": cc,
            "ones_bd": ones_bd,
        }
        for i in range(NCORES)
    ]


def kernel(x, mu, log_sigma, log_alpha):
    if "nc" not in _CACHE:
        _CACHE["nc"] = _build_nc()
    nc = _CACHE["nc"]
    in_maps = _in_maps(x, mu, log_sigma, log_alpha)
    res = run_bass_kernel_spmd(nc, in_maps, list(range(NCORES))).results
    outs = []
    for i in range(NCORES):
        dev = np.asarray(res[i]["out"])  # [BPC, 2K, N//2]
        # partition 64h+k, column 1024g+512p+c  ->  [k, 2048g+1024p+512h+c]
        v = dev.reshape(BPC, 2, K, NG, 2, NT)
        outs.append(
            np.transpose(v, (0, 2, 3, 4, 1, 5)).reshape(BPC, K, N)
        )
    return np.concatenate(outs, axis=0).astype(np.float32)
